# revision 4
# baseline (speedup 1.0000x reference)
"""Causal self-attention (B=4, S=2048, D=768, H=12) on 8 TRN2 NeuronCores. v2.

Sharding: batch (4) x head-group (2) = 8 cores; each core does its batch's
6 heads end-to-end plus its 384 rows of W_out; host sums the two partial
outputs per batch and adds the constant b_v @ W_out + b_out.

v2 changes vs v1:
  - bf16 operands everywhere on the PE (fp32 PSUM accumulation); host casts.
  - x^T via XBAR DMA-transpose straight from DRAM (no PE transposes, no
    PSUM staging, no eviction copies).
  - t-major pair-interleaved schedule: attention tiles for all 3 head-pairs
    run per q-window, with the next window's QK/V projections and the
    previous window's output projection interleaved as PE filler between
    attention chunks (keeps the PE warm => 2.4GHz HAM state).
  - software-pipelined chunk loop: AV matmuls lag scores by one chunk so
    the PE never waits on the ACT exp.
  - causal diagonal processed at widths 512/384/256/128 (bf16 is full-rate
    at any width) with a single shared [128,128] triangle mask applied on
    the GPSIMD engine (frees DVE).
  - output projection written to DRAM straight from PSUM (f32, no evict).
"""

import numpy as np

import concourse.bass as bass
import concourse.tile as tile
import concourse.mybir as mybir
from concourse import bacc
from concourse._compat import with_exitstack  # noqa: F401

F32 = mybir.dt.float32
F32R = mybir.dt.float32r
BF16 = mybir.dt.bfloat16

B, S, D = 4, 2048, 768
H, DH = 12, 64
G = 2
HPG = H // G          # 6
NPAIR = HPG // 2      # 3
N_CORES = 8
ST = 128
QT = 512
KC = 128
N_ST = S // ST        # 16
N_QT = S // QT        # 4
DC = D // 128         # 6
EXP = mybir.ActivationFunctionType.Exp


def declare_io(nc):
    io = {}
    io["x"] = nc.dram_tensor("x", [S, D], BF16, kind="ExternalInput")
    io["wqk"] = nc.dram_tensor("wqk", [D, 768], BF16, kind="ExternalInput")
    io["bqk2"] = nc.dram_tensor("bqk2", [128, 6], F32, kind="ExternalInput")
    io["wv"] = nc.dram_tensor("wv", [D, 384], BF16, kind="ExternalInput")
    io["wo"] = nc.dram_tensor("wo", [384, 768], BF16, kind="ExternalInput")
    io["mask"] = nc.dram_tensor("mask", [KC, KC], BF16, kind="ExternalInput")
    io["sel"] = nc.dram_tensor("sel", [128, 128], F32R, kind="ExternalInput")
    io["ones2"] = nc.dram_tensor("ones2", [128, HPG], BF16, kind="ExternalInput")
    io["out"] = nc.dram_tensor("out", [S, D], BF16, kind="ExternalOutput")
    return io


def build_body(nc, tc, pools, io):
    (consts, wqk_p, wv_p, wo_p, xT_p, qkT_p, vsb_p, pt_p, apair_p,
     rc_p, bcsb_p, at_p, outsb_p, psP, scp, psB) = pools

    # ---- constants ----
    mask_t = consts.tile([KC, KC], BF16, tag="mask")
    nc.sync.dma_start(out=mask_t, in_=io["mask"][:])
    sel_t = consts.tile([128, 128], F32R, tag="sel")
    nc.sync.dma_start(out=sel_t, in_=io["sel"][:])
    bqk2_t = consts.tile([128, 6], F32, tag="bqk2")
    nc.sync.dma_start(out=bqk2_t, in_=io["bqk2"][:])

    # ---- weights ----
    wqk_t = []
    for c in range(DC):
        w = wqk_p.tile([128, 768], BF16, tag="wqk", name=f"wqk{c}")
        nc.sync.dma_start(out=w, in_=io["wqk"][c * 128:(c + 1) * 128, :])
        wqk_t.append(w)
    wv_t = []
    for c in range(DC):
        w = wv_p.tile([128, 384], BF16, tag="wv", name=f"wv{c}")
        nc.scalar.dma_start(out=w, in_=io["wv"][c * 128:(c + 1) * 128, :])
        wv_t.append(w)
    wo_t = []
    for p in range(NPAIR):
        w = wo_p.tile([128, 768], BF16, tag="wo", name=f"wo{p}")
        nc.scalar.dma_start(out=w, in_=io["wo"][p * 128:(p + 1) * 128, :])
        wo_t.append(w)

    # ---- x^T via XBAR DMA transpose (split on S for pipelining) ----
    # NOTE: all transposes issue from ONE HWDGE queue (SP). Concurrent XBAR
    # transposes from two queues corrupt each other on HW (verified).
    xT = [xT_p.tile([128, S], BF16, tag="xT", name=f"xT{c}") for c in range(DC)]
    for h in range(2):
        for c in range(DC):
            nc.sync.dma_start_transpose(
                xT[c][:, h * 1024:(h + 1) * 1024],
                io["x"][h * 1024:(h + 1) * 1024, c * 128:(c + 1) * 128])

    qkT = [qkT_p.tile([128, S], BF16, tag="qkT", name=f"qkT{j}") for j in range(6)]
    vsb = [None] * N_ST
    apair = [apair_p.tile([128, S], BF16, tag="apair", name=f"apair{p}")
             for p in range(NPAIR)]

    # ---- emission units ----
    def emit_qkproj(j, t):
        pp = psP.tile([128, QT], F32, tag="psP")
        for c in range(DC):
            nc.tensor.matmul(pp, wqk_t[c][:, j * 128:(j + 1) * 128],
                             xT[c][:, t * QT:(t + 1) * QT],
                             start=(c == 0), stop=(c == DC - 1))
        nc.vector.tensor_scalar_add(qkT[j][:, t * QT:(t + 1) * QT], pp,
                                    bqk2_t[:, j:j + 1])

    def emit_vproj(s):
        vp = psP.tile([128, 384], F32, tag="psP")
        for c in range(DC):
            nc.tensor.matmul(vp, xT[c][:, s * ST:(s + 1) * ST], wv_t[c][:],
                             start=(c == 0), stop=(c == DC - 1))
        vv = vsb_p.tile([128, HPG, 65], BF16, tag="vsb")
        nc.vector.tensor_copy(vv[:, :, 0:64],
                              vp[:].rearrange("p (h d) -> p h d", h=HPG))
        nc.sync.dma_start(out=vv[:, :, 64:65],
                          in_=io["ones2"][:].rearrange("p (h o) -> p h o", o=1))
        vsb[s] = vv

    def emit_outproj(s):
        o1 = psP.tile([128, 512], F32, tag="psP")
        o2 = psP.tile([128, 256], F32, tag="psP")
        for p in range(NPAIR):
            nc.tensor.matmul(o1, apair[p][:, s * ST:(s + 1) * ST],
                             wo_t[p][:, 0:512],
                             start=(p == 0), stop=(p == NPAIR - 1))
        for p in range(NPAIR):
            nc.tensor.matmul(o2, apair[p][:, s * ST:(s + 1) * ST],
                             wo_t[p][:, 512:768],
                             start=(p == 0), stop=(p == NPAIR - 1))
        osb = outsb_p.tile([128, D], BF16, tag="outsb")
        nc.vector.tensor_copy(osb[:, 0:512], o1)
        nc.scalar.copy(osb[:, 512:768], o2)
        nc.sync.dma_start(out=io["out"][s * ST:(s + 1) * ST, :], in_=osb)

    # broadcast middle-dim view of the triangle mask: [128, 2, 128]
    mask2 = bass.AP(tensor=mask_t.tensor, offset=mask_t.offset,
                    ap=[list(mask_t.ap[0]), [0, 2], list(mask_t.ap[1])])

    def attn_tile(p, t, fillers):
        qp = qkT[2 * p]
        kp = qkT[2 * p + 1]
        n_kc = 4 * t + 4
        av_e = psB.tile([65, QT], F32, tag="psB")
        av_o = psB.tile([65, QT], F32, tag="psB")
        avs = (av_e, av_o)
        fq = list(fillers)

        def chunk_geom(kc):
            r = kc - 4 * t
            if r < 0:
                return 0, QT
            if r == 3:
                return 384, 128
            return 128 * r, QT - 128 * r

        def emit_av(pend):
            pt2_, kc_, off_, w_ = pend
            for j in (0, 1):
                nc.tensor.matmul(
                    avs[j][:, off_:off_ + w_], vsb[kc_][:, 2 * p + j, :],
                    pt2_[:, j, 0:w_],
                    start=(kc_ == 0), stop=(kc_ == n_kc - 1))

        pending = None
        for kc in range(n_kc):
            off, w = chunk_geom(kc)
            sc2 = scp.tile([KC, 2, QT], F32, tag="sc2")
            for j in (0, 1):
                nc.tensor.matmul(
                    sc2[:, j, 0:w],
                    kp[j * 64:(j + 1) * 64, kc * KC:(kc + 1) * KC],
                    qp[j * 64:(j + 1) * 64, t * QT + off:t * QT + off + w],
                    start=True, stop=True, tile_position=(j * 64, 0))
            if pending is not None:
                emit_av(pending)
            pt2 = pt_p.tile([KC, 2, QT], BF16, tag="pt2")
            nc.scalar.activation(pt2[:, :, 0:w], sc2[:, :, 0:w], EXP)
            if kc - 4 * t >= 0:
                nc.gpsimd.tensor_mul(pt2[:, :, 0:128], pt2[:, :, 0:128], mask2)
            pending = (pt2, kc, off, w)
            if kc % 2 == 1 and fq:
                fq.pop(0)()
        emit_av(pending)
        for f in fq:
            f()

        # ---- normalization: O /= den  (den rides row 64 of av via ones col)
        cols = slice(t * QT, (t + 1) * QT)
        rc_e = rc_p.tile([65, QT], F32R, tag="rc")
        nc.vector.reciprocal(rc_e[64:65, :], av_e[64:65, :])
        bc_e = psP.tile([64, QT], F32, tag="psP")
        nc.tensor.matmul(bc_e, sel_t[64:65, 0:64], rc_e[64:65, :],
                         start=True, stop=True)
        bc_e_sb = bcsb_p.tile([64, QT], F32R, tag="bcsb")
        nc.vector.tensor_copy(bc_e_sb, bc_e)
        nc.vector.tensor_mul(apair[p][0:64, cols], av_e[0:64, :], bc_e_sb[:])

        rc_o = rc_p.tile([65, QT], F32R, tag="rc")
        nc.vector.reciprocal(rc_o[64:65, :], av_o[64:65, :])
        bc_o = psP.tile([64, QT], F32, tag="psP")
        nc.tensor.matmul(bc_o, sel_t[64:65, 0:64], rc_o[64:65, :],
                         start=True, stop=True)
        bc_o_sb = bcsb_p.tile([64, QT], F32R, tag="bcsb")
        nc.vector.tensor_copy(bc_o_sb, bc_o)
        at = at_p.tile([64, QT], BF16, tag="at")
        nc.vector.tensor_mul(at, av_o[0:64, :], bc_o_sb[:])
        nc.sync.dma_start(out=apair[p][64:128, cols], in_=at)

    # ---- schedule ----
    for s in range(4):
        emit_vproj(s)
    emit_qkproj(0, 0)
    emit_qkproj(1, 0)

    def qk(j, t):
        return lambda: emit_qkproj(j, t)

    def vp(s):
        return lambda: emit_vproj(s)

    def op(s):
        return lambda: emit_outproj(s)

    plan = {
        (0, 0): [qk(2, 0), qk(3, 0)],
        (0, 1): [qk(4, 0), qk(5, 0)],
        (0, 2): [vp(4), vp(5), vp(6), vp(7), qk(0, 1), qk(1, 1)],
        (1, 0): [op(0), qk(2, 1), qk(3, 1)],
        (1, 1): [op(1), qk(4, 1), qk(5, 1)],
        (1, 2): [op(2), op(3), vp(8), vp(9), vp(10), vp(11), qk(0, 2), qk(1, 2)],
        (2, 0): [op(4), qk(2, 2), qk(3, 2)],
        (2, 1): [op(5), qk(4, 2), qk(5, 2)],
        (2, 2): [op(6), op(7), vp(12), vp(13), vp(14), vp(15), qk(0, 3), qk(1, 3)],
        (3, 0): [op(8), qk(2, 3), qk(3, 3)],
        (3, 1): [op(9), qk(4, 3), qk(5, 3)],
        (3, 2): [op(10), op(11)],
    }
    for t in range(N_QT):
        for p in range(NPAIR):
            attn_tile(p, t, plan[(t, p)])
    for s in range(12, 16):
        emit_outproj(s)


def make_pools(tc, ctx):
    consts = ctx.enter_context(tc.tile_pool(name="consts", bufs=1))
    wqk_p = ctx.enter_context(tc.tile_pool(name="wqk", bufs=12))
    wv_p = ctx.enter_context(tc.tile_pool(name="wv", bufs=12))
    wo_p = ctx.enter_context(tc.tile_pool(name="wo", bufs=6))
    xT_p = ctx.enter_context(tc.tile_pool(name="xT", bufs=12))
    qkT_p = ctx.enter_context(tc.tile_pool(name="qkT", bufs=12))
    vsb_p = ctx.enter_context(tc.tile_pool(name="vsb", bufs=20))
    pt_p = ctx.enter_context(tc.tile_pool(name="pt", bufs=3))
    apair_p = ctx.enter_context(tc.tile_pool(name="apair", bufs=3))
    rc_p = ctx.enter_context(tc.tile_pool(name="rc", bufs=4))
    bcsb_p = ctx.enter_context(tc.tile_pool(name="bcsb", bufs=4))
    at_p = ctx.enter_context(tc.tile_pool(name="at", bufs=2))
    outsb_p = ctx.enter_context(tc.tile_pool(name="outsb", bufs=2))
    psP = ctx.enter_context(tc.tile_pool(name="psP", bufs=2, space="PSUM"))
    scp = ctx.enter_context(tc.tile_pool(name="scp", bufs=2, space="PSUM"))
    psB = ctx.enter_context(tc.tile_pool(name="psB", bufs=2, space="PSUM"))
    return (consts, wqk_p, wv_p, wo_p, xT_p, qkT_p, vsb_p, pt_p, apair_p,
            rc_p, bcsb_p, at_p, outsb_p, psP, scp, psB)


def build_nc(n_iters=None):
    from contextlib import ExitStack

    nc = bacc.Bacc(trn_type="TRN2", debug=False)
    nc._allow_low_precision_reason = "bf16 operands; fp32 PSUM accumulation"
    io = declare_io(nc)
    with tile.TileContext(nc) as tc:
        with ExitStack() as ctx:
            pools = make_pools(tc, ctx)
            if n_iters is None:
                build_body(nc, tc, pools, io)
            else:
                with tc.For_i(0, n_iters, 1):
                    build_body(nc, tc, pools, io)
    nc.compile()
    return nc, io


def host_inputs(x, W_qkv, b_qkv, W_out, b_out):
    import ml_dtypes
    bf16 = ml_dtypes.bfloat16

    x = np.asarray(x, dtype=np.float32)
    W_qkv = np.asarray(W_qkv, dtype=np.float32)
    b_qkv = np.asarray(b_qkv, dtype=np.float32)
    W_out = np.asarray(W_out, dtype=np.float32)
    b_out = np.asarray(b_out, dtype=np.float32)

    Wq, Wk, Wv = W_qkv[:, 0:D], W_qkv[:, D:2 * D], W_qkv[:, 2 * D:3 * D]
    bq, bk, bv = b_qkv[0:D], b_qkv[D:2 * D], b_qkv[2 * D:3 * D]
    scale = 1.0 / np.sqrt(DH)

    mask = np.tril(np.ones((KC, KC), np.float32)).T.astype(bf16)  # q >= k
    sel = np.zeros((128, 128), np.float32)
    sel[64, 0:64] = 1.0
    ones2 = np.ones((128, HPG), np.float32).astype(bf16)

    per_group = []
    for g in range(G):
        cols = []
        bcols = []
        for p in range(NPAIR):
            h0 = g * HPG + 2 * p
            cols.append(Wq[:, h0 * DH:(h0 + 2) * DH] * scale)
            cols.append(Wk[:, h0 * DH:(h0 + 2) * DH])
            bcols.append(bq[h0 * DH:(h0 + 2) * DH] * scale)
            bcols.append(bk[h0 * DH:(h0 + 2) * DH])
        wqk_g = np.concatenate(cols, axis=1).astype(bf16)        # [768, 768]
        bqk_g = np.stack(bcols, axis=1).astype(np.float32)       # [128, 6]
        wv_g = Wv[:, g * HPG * DH:(g + 1) * HPG * DH].astype(bf16)
        wo_g = W_out[g * HPG * DH:(g + 1) * HPG * DH, :].astype(bf16)
        per_group.append((wqk_g, bqk_g, wv_g, wo_g))

    xb = x.astype(bf16)
    in_maps = []
    for core in range(N_CORES):
        b, g = core // G, core % G
        wqk_g, bqk_g, wv_g, wo_g = per_group[g]
        in_maps.append(dict(
            x=np.ascontiguousarray(xb[b]),
            wqk=np.ascontiguousarray(wqk_g),
            bqk2=np.ascontiguousarray(bqk_g),
            wv=np.ascontiguousarray(wv_g),
            wo=np.ascontiguousarray(wo_g),
            mask=mask, sel=sel, ones2=ones2,
        ))
    cvec = (bv @ W_out + b_out).astype(np.float32)
    return in_maps, cvec


_CACHE = {}


def kernel(x, W_qkv, b_qkv, W_out, b_out):
    from concourse.bass_utils import run_bass_kernel_spmd

    if "nc" not in _CACHE:
        _CACHE["nc"], _ = build_nc()
    nc = _CACHE["nc"]
    in_maps, cvec = host_inputs(x, W_qkv, b_qkv, W_out, b_out)
    res = run_bass_kernel_spmd(nc, in_maps, list(range(N_CORES)))
    out = np.empty((B, S, D), np.float32)
    for b in range(B):
        out[b] = (np.asarray(res.results[2 * b]["out"], np.float32)
                  + np.asarray(res.results[2 * b + 1]["out"], np.float32)
                  + cvec)
    return out
